# revision 8
# baseline (speedup 1.0000x reference)
"""GCN (2-layer + linear head + log_softmax) on 8 Trainium2 NeuronCores.

Strategy (graph/data parallel, per sharding hint):
  - Nodes partitioned across 8 cores (degree-sorted serpentine), weights
    replicated.  Per GCN layer each core computes h = x_shard @ W on PE,
    scales rows by dinv = (deg+1)^-1/2 and AllGathers the scaled table in
    2 window chunks of PAIR rows: one 256B table row holds the features
    of two nodes (adjacent feature groups 2G,2G+1 at the same partition),
    so the exchange ships each node's 128B of features exactly once.
  - Aggregation: batched SWDGE dma_gather (mlp gpsimd library) pulls
    per-edge 256B pair rows (int16 window-local indices, 4 SWDGE queues
    round-robin) into SBUF column tiles; PE one-hot "segment matmuls"
    reduce each 128-slot column into per-destination partial sums in
    PSUM.  Each column gets two matmuls - parity A (even source group,
    first 128B of the row) and parity B (odd, second 128B) - with masks
    M[e, rank] = [segid_par[e] == rank] built by DVE from per-parity
    segid tables; a sentinel segid kills padding slots, so padding
    indices are spread over the table (a single shared zero row would
    hotspot one HBM line and serialize the SDMA engines).
  - Math identity:  out[d] = dinv[d] * sum_{e:dst=d} dinv[src]*h[src]
                             + dinv[d]^2 * h[d] + b
    so the gathered table is pre-scaled by dinv and no per-edge
    coefficients are needed.

Host-side numpy does only graph-structure preprocessing (degree counting,
node->core/rank assignment, gather-index/segment-id construction) and the
output unpermute.  All floating-point tensor math runs on the NeuronCores.
"""

import os

import numpy as np

import concourse.bass as bass
import concourse.bacc as bacc
import concourse.mybir as mybir
import concourse.tile as tile
from concourse.bass_utils import run_bass_kernel_spmd
from concourse.masks import make_identity
from concourse import library_config

FP16 = mybir.dt.float16
F32 = mybir.dt.float32
I16 = mybir.dt.int16

N_CORES = 8
P = 128           # partitions
F_DIM = 64        # in = hidden = 64
C_DIM = 16
NG = 98           # feature groups per core (SHR / 128)
SHR = 12544       # node slots per core (128 * 98)
W = 2             # gather windows (int16 index range)
WP = 64           # partitions per window (128 / 2)
PG = NG // 2      # pair groups per core (49): row = groups (2G, 2G+1)
WG = WP * PG      # pair rows contributed per core per window = 3136
WROWS = N_CORES * WG   # pair rows per window table = 25088
RG = 8            # dst groups per PSUM range
NR = (NG + RG - 1) // RG   # 13 ranges
CAP = int(os.environ.get("KCAP", "16"))   # max gather columns per call
                  # (small calls round-robined over 4 SWDGE queues)
SEG_PAD = 200.0   # segid sentinel for padding slots (matches no rank)


class _Call:
    __slots__ = ("w", "ncols", "off16", "col_off", "mms")


class _Plan:
    pass


def build_plan(edge_index, n_nodes):
    """Host-side graph preprocessing.  Pure index math, O(E log E)."""
    src = np.asarray(edge_index[0], dtype=np.int64)
    dst = np.asarray(edge_index[1], dtype=np.int64)
    E = src.shape[0]

    deg = np.bincount(dst, minlength=n_nodes).astype(np.int64)

    # serpentine degree-desc core assignment
    order = np.argsort(-deg, kind="stable")
    pos = np.arange(n_nodes)
    blk, lane = pos // N_CORES, pos % N_CORES
    core_of_pos = np.where(blk % 2 == 0, lane, N_CORES - 1 - lane)
    nodes_per_core = [order[core_of_pos == c] for c in range(N_CORES)]
    n_max = max(len(v) for v in nodes_per_core)
    assert n_max <= SHR, "shard overflow"

    usable = np.arange(SHR, dtype=np.int64)

    rank_of = np.full(n_nodes, -1, dtype=np.int64)
    core_of = np.full(n_nodes, -1, dtype=np.int64)
    for c, nl in enumerate(nodes_per_core):
        rank_of[nl] = usable[: len(nl)]
        core_of[nl] = c

    p_of = rank_of % P
    g_of = rank_of // P
    w_of = p_of // WP
    par_of = g_of % 2
    # window-local pair-row of a node
    loc_of = core_of * WG + (p_of % WP) * PG + g_of // 2

    # per-core per (dst-group, src-window) edge counts
    cnt = np.zeros((N_CORES, NG, W), dtype=np.int64)
    ecore = core_of[dst]
    for c in range(N_CORES):
        m = ecore == c
        np.add.at(cnt[c], (g_of[dst[m]], w_of[src[m]]), 1)

    # shared (max over cores) column counts per (g, w); >=1 so every group
    # appears in window 0 (s init via copy) and DVE accum stays coarse
    cols_gw = np.maximum(1, -(-cnt.max(axis=0) // P))   # [NG, W]

    # global column offsets, order (w, range, g)
    o_gw = np.zeros((NG, W), dtype=np.int64)
    col = 0
    rw_list = []   # (w, R, rgw, [calls])
    for w in range(W):
        for R in range(NR):
            g0, g1 = R * RG, min(NG, R * RG + RG)
            span0 = col
            for g in range(g0, g1):
                o_gw[g, w] = col
                col += int(cols_gw[g, w])
            # split span into calls of <= CAP columns
            calls = []
            c0 = span0
            while c0 < col:
                c1 = min(col, c0 + CAP)
                call = _Call()
                call.w = w
                call.ncols = c1 - c0
                call.off16 = c0 * 8          # slot offset / 16
                call.col_off = c0
                call.mms = []
                calls.append(call)
                c0 = c1
            # per-column matmul descriptors: two per column (parity A/B)
            for g in range(g0, g1):
                kc = int(cols_gw[g, w])
                for k in range(kc):
                    cg = int(o_gw[g, w]) + k
                    for call in calls:
                        if call.col_off <= cg < call.col_off + call.ncols:
                            ci = cg - call.col_off
                            gl = g - g0
                            for par in (0, 1):
                                call.mms.append(
                                    (ci, gl, par,
                                     k == 0 and par == 0,
                                     k == kc - 1 and par == 1))
                            break
            rw_list.append((w, R, g1 - g0, calls))
    TOT_COLS = col
    TOT_SLOTS = TOT_COLS * P
    maxcall = max(c.ncols for (_, _, _, cl) in rw_list for c in cl)

    # per-core gather indices + per-parity segment ids
    idx16 = np.empty((N_CORES, P, TOT_SLOTS // 16), dtype=np.int16)
    segid = np.empty((N_CORES, P, 2 * TOT_COLS), dtype=np.float16)
    for c in range(N_CORES):
        m = ecore == c
        d_g = g_of[dst[m]]
        d_p = p_of[dst[m]]
        s_w = w_of[src[m]]
        s_loc = loc_of[src[m]]
        s_par = par_of[src[m]]
        key = d_g * W + s_w
        o = np.argsort(key, kind="stable")
        key_s = key[o]
        first = np.searchsorted(key_s, key_s, side="left")
        k = np.arange(len(key_s)) - first
        colno = o_gw[d_g[o], s_w[o]] + k // P
        part = k % P
        slot = colno * P + part
        # padding slots gather garbage rows the sentinel mask zeroes out;
        # spread them so no single 256B HBM line hotspots the SDMA engines
        idx_flat = ((np.arange(TOT_SLOTS) * 97) % WROWS).astype(np.int16)
        segAB = np.full((TOT_SLOTS, 2), SEG_PAD, dtype=np.float16)
        idx_flat[slot] = s_loc[o].astype(np.int16)
        segAB[slot, s_par[o]] = d_p[o].astype(np.float16)
        wrapped = idx_flat.reshape(TOT_SLOTS // 16, 16).T   # [16, S/16]
        idx16[c] = np.tile(wrapped, (8, 1))
        # [P, 2*TOT_COLS], parity interleaved per column
        segid[c] = segAB.reshape(TOT_COLS, P, 2).transpose(1, 0, 2).reshape(
            P, 2 * TOT_COLS)

    plan = _Plan()
    plan.n_nodes = n_nodes
    plan.E = E
    plan.TOT_COLS = TOT_COLS
    plan.TOT_SLOTS = TOT_SLOTS
    plan.maxcall = maxcall
    plan.rw_list = rw_list
    plan.nodes_per_core = nodes_per_core
    plan.rank_of = rank_of
    plan.usable = usable
    plan.idx16 = idx16
    plan.segid = segid
    plan.deg = deg
    return plan


def build_inputs(plan, x, W1, b1, W2, b2, Wl, bl):
    """Per-core input dicts for run_bass_kernel_spmd."""
    in_maps = []
    W1h = np.ascontiguousarray(W1.astype(np.float16))
    W2h = np.ascontiguousarray(W2.astype(np.float16))
    Wlh = np.ascontiguousarray(Wl.astype(np.float16))
    b1r = np.ascontiguousarray(np.broadcast_to(b1.astype(np.float16), (P, F_DIM)))
    b2r = np.ascontiguousarray(np.broadcast_to(b2.astype(np.float16), (P, F_DIM)))
    blr = np.ascontiguousarray(np.broadcast_to(bl.astype(np.float32), (P, C_DIM)))
    iota = np.ascontiguousarray(
        np.broadcast_to(np.arange(P, dtype=np.float16), (P, P)))
    for c in range(N_CORES):
        nl = plan.nodes_per_core[c]
        ranks = plan.usable[: len(nl)]
        xT = np.zeros((F_DIM, SHR), dtype=np.float16)
        xT[:, ranks] = np.asarray(x)[nl].astype(np.float16).T
        degp1 = np.full((P, NG), 1e30, dtype=np.float32)
        degp1[ranks % P, ranks // P] = (plan.deg[nl] + 1).astype(np.float32)
        in_maps.append({
            "xT": xT,
            "degp1": degp1,
            "gidx": np.ascontiguousarray(plan.idx16[c]),
            "segid": np.ascontiguousarray(plan.segid[c]),
            "iota": iota,
            "W1": W1h, "W2": W2h, "Wl": Wlh,
            "b1r": b1r, "b2r": b2r, "blr": blr,
        })
    return in_maps


def build_bass(plan, repeat=1):
    NF = NG * F_DIM
    TOT16 = plan.TOT_SLOTS // 16
    MC = plan.maxcall
    nc = bacc.Bacc("TRN2", target_bir_lowering=False, debug=False,
                   num_devices=N_CORES, dynamic_dma_scratch_size=16384,
                   num_swdge_queues=4)

    xT_d = nc.dram_tensor("xT", [F_DIM, SHR], FP16, kind="ExternalInput")
    degp1_d = nc.dram_tensor("degp1", [P, NG], F32, kind="ExternalInput")
    gidx_d = nc.dram_tensor("gidx", [P, TOT16], I16, kind="ExternalInput")
    segid_d = nc.dram_tensor("segid", [P, 2 * plan.TOT_COLS], FP16,
                             kind="ExternalInput")
    iota_d = nc.dram_tensor("iota", [P, P], FP16, kind="ExternalInput")
    W1_d = nc.dram_tensor("W1", [F_DIM, F_DIM], FP16, kind="ExternalInput")
    W2_d = nc.dram_tensor("W2", [F_DIM, F_DIM], FP16, kind="ExternalInput")
    Wl_d = nc.dram_tensor("Wl", [F_DIM, C_DIM], FP16, kind="ExternalInput")
    b1r_d = nc.dram_tensor("b1r", [P, F_DIM], FP16, kind="ExternalInput")
    b2r_d = nc.dram_tensor("b2r", [P, F_DIM], FP16, kind="ExternalInput")
    blr_d = nc.dram_tensor("blr", [P, C_DIM], F32, kind="ExternalInput")
    y_d = nc.dram_tensor("y", [SHR, C_DIM], F32, kind="ExternalOutput")

    gsh_all = [[[nc.dram_tensor(f"gsh{l}_{w}_{r}", [WG, 2 * F_DIM], FP16)
                 for w in range(W)] for l in range(2)] for r in range(repeat)]
    gfull_all = [[[nc.dram_tensor(f"gfull{l}_{w}_{r}", [WROWS, 2 * F_DIM],
                                  FP16, addr_space="Shared")
                   for w in range(W)] for l in range(2)] for r in range(repeat)]

    rg = [list(range(N_CORES))]

    with tile.TileContext(nc) as tc:
        with (
            tc.tile_pool(name="const", bufs=1) as constp,
            tc.tile_pool(name="persist", bufs=1) as pers,
            tc.tile_pool(name="work", bufs=2) as workp,
            tc.tile_pool(name="xt", bufs=2) as xtp,
            tc.tile_pool(name="gath", bufs=int(os.environ.get("KGB", "8"))) as gathp,
            tc.tile_pool(name="mm", bufs=int(os.environ.get("KMB", "4"))) as mp,
            tc.tile_pool(name="psum", bufs=2, space="PSUM") as psump,
        ):
            # Load the mlp gpsimd library (dma_gather) up front: a reload
            # auto-inserted mid-program next to in-flight collectives kills
            # the Q7 cores (NRT_EXEC_UNIT_UNRECOVERABLE).
            nc.gpsimd.load_library(library_config.mlp)

            # ---- constants ----
            W1_sb = constp.tile([F_DIM, F_DIM], FP16, tag="W1")
            nc.sync.dma_start(out=W1_sb, in_=W1_d[:, :])
            W2_sb = constp.tile([F_DIM, F_DIM], FP16, tag="W2")
            nc.sync.dma_start(out=W2_sb, in_=W2_d[:, :])
            Wl_sb = constp.tile([F_DIM, C_DIM], FP16, tag="Wl")
            nc.sync.dma_start(out=Wl_sb, in_=Wl_d[:, :])
            b1_sb = constp.tile([P, F_DIM], FP16, tag="b1")
            nc.sync.dma_start(out=b1_sb, in_=b1r_d[:, :])
            b2_sb = constp.tile([P, F_DIM], FP16, tag="b2")
            nc.sync.dma_start(out=b2_sb, in_=b2r_d[:, :])
            bl_sb = constp.tile([P, C_DIM], F32, tag="bl")
            nc.sync.dma_start(out=bl_sb, in_=blr_d[:, :])
            ident = constp.tile([P, P], FP16, tag="ident")
            make_identity(nc, ident[:, :])
            iota_sb = constp.tile([P, P], FP16, tag="iota")
            nc.sync.dma_start(out=iota_sb, in_=iota_d[:, :])

            idx_sb = constp.tile([P, TOT16], I16, tag="idx")
            nc.sync.dma_start(out=idx_sb, in_=gidx_d[:, :])
            segid_sb = constp.tile([P, 2 * plan.TOT_COLS], FP16, tag="segid")
            nc.sync.dma_start(out=segid_sb, in_=segid_d[:, :])

            # ---- dinv ----
            degp1_sb = constp.tile([P, NG], F32, tag="degp1")
            nc.sync.dma_start(out=degp1_sb, in_=degp1_d[:, :])
            rec_sb = constp.tile([P, NG], F32, tag="rec")
            nc.vector.reciprocal(rec_sb[:, :], degp1_sb[:, :])
            dinv_sb = constp.tile([P, NG], F32, tag="dinv")
            nc.scalar.activation(dinv_sb[:, :], rec_sb[:, :],
                                 mybir.ActivationFunctionType.Sqrt)
            dinv_rep = constp.tile([P, NF], FP16, tag="dinvrep")
            nc.vector.tensor_copy(
                dinv_rep[:, :].rearrange("p (g f) -> p g f", g=NG, f=F_DIM),
                dinv_sb[:, :].unsqueeze(2).broadcast_to([P, NG, F_DIM]),
            )

            def dense_matmul_stream(W_sb, out_tag):
                """layer-1 shard matmul, xT streamed from DRAM."""
                out_sb = pers.tile([P, NF], FP16, tag=out_tag)
                per_ps = 8
                for blk0 in range(0, NG, per_ps):
                    blk1 = min(NG, blk0 + per_ps)
                    nb = blk1 - blk0
                    xt = xtp.tile([F_DIM, per_ps * P], FP16, tag="xt")
                    nc.sync.dma_start(out=xt[:, :nb * P],
                                      in_=xT_d[:, blk0 * P: blk1 * P])
                    ps = psump.tile([P, 512], F32, tag="mmps")
                    for g in range(blk0, blk1):
                        kk = g - blk0
                        nc.tensor.matmul(
                            ps[:, kk * F_DIM:(kk + 1) * F_DIM],
                            lhsT=xt[:, kk * P: kk * P + P],
                            rhs=W_sb[:, :], start=True, stop=True)
                    nc.scalar.activation(
                        out_sb[:, blk0 * F_DIM: blk0 * F_DIM + nb * F_DIM],
                        ps[:, :nb * F_DIM],
                        mybir.ActivationFunctionType.Copy)
                return out_sb

            def dense_matmul_nodemajor(h_sb, W_sb, n_out, out_tag,
                                       out_dtype=FP16):
                """input node-major [128, NG*64]: PE-transpose 4 groups at a
                time, then matmul."""
                out_sb = pers.tile([P, NG * n_out], out_dtype, tag=out_tag)
                for blk0 in range(0, NG, 4):
                    blk1 = min(NG, blk0 + 4)
                    nb = blk1 - blk0
                    tps = psump.tile([F_DIM, 512], FP16, tag="tps")
                    for g in range(blk0, blk1):
                        kk = g - blk0
                        nc.tensor.transpose(
                            tps[:, kk * P: kk * P + P],
                            in_=h_sb[:, g * F_DIM: (g + 1) * F_DIM],
                            identity=ident[:, :],
                        )
                    hTt = workp.tile([F_DIM, 512], FP16, tag="hTt")
                    nc.scalar.activation(hTt[:, :nb * P], tps[:, :nb * P],
                                         mybir.ActivationFunctionType.Copy)
                    ps = psump.tile([P, 512], F32, tag="mmps")
                    for g in range(blk0, blk1):
                        kk = g - blk0
                        nc.tensor.matmul(
                            ps[:, kk * n_out:(kk + 1) * n_out],
                            lhsT=hTt[:, kk * P: kk * P + P],
                            rhs=W_sb[:, :], start=True, stop=True)
                    nc.scalar.activation(
                        out_sb[:, blk0 * n_out: blk0 * n_out + nb * n_out],
                        ps[:, :nb * n_out],
                        mybir.ActivationFunctionType.Copy)
                return out_sb

            def mul_rep(h_sb, tag):
                o = pers.tile([P, NF], FP16, tag=tag)
                nc.vector.tensor_tensor(o[:, :], h_sb[:, :], dinv_rep[:, :],
                                        op=mybir.AluOpType.mult)
                return o

            def cc_observe(gfull_t):
                # tiny SWDGE read carries the collective-done wait once
                obs = constp.tile([1, F_DIM], FP16, tag="ccobs")
                nc.gpsimd.dma_start(out=obs[:, :], in_=gfull_t[0:1, 0:F_DIM])

            KCC = int(os.environ.get("KCC", "1"))

            def table_exchange(g_sb, gsh, gfull):
                """write dinv-scaled shard as pair rows (2 nodes / 256B) and
                AllGather per window.  Pair layout is free: groups 2G,2G+1
                are adjacent in the free dim of g_sb."""
                for w in range(W):
                    nc.sync.dma_start(
                        out=gsh[w][:, :].rearrange("(q G) f -> q (G f)", q=WP),
                        in_=g_sb[WP * w: WP * (w + 1), :])
                    if not KCC:
                        continue
                    nc.gpsimd.collective_compute(
                        "AllGather", mybir.AluOpType.bypass,
                        replica_groups=rg,
                        ins=[gsh[w][:, :].opt()], outs=[gfull[w][:, :].opt()],
                    )
                    cc_observe(gfull[w])

            AGG_MODE = int(os.environ.get("KAGG", "3"))

            gq = [0]  # global gather queue round-robin

            def aggregate(gfull, out_tag):
                s_sb = pers.tile([P, NF], FP16, tag=out_tag)
                if AGG_MODE == 0:
                    nc.vector.memset(s_sb[:, :], 0.0)
                    return s_sb
                for (w, R, rgw, calls) in plan.rw_list:
                    if AGG_MODE >= 3:
                        ps = psump.tile([P, 512], F32, tag="agg")
                    for call in calls:
                        ncol = call.ncols
                        gt = gathp.tile([P, MC * P], FP16, tag="gt")
                        nc.gpsimd.dma_gather(
                            out_ap=gt[:, :ncol * P].rearrange(
                                "p (c e) -> p c e", c=ncol, e=P),
                            in_ap=gfull[w][:, :],
                            idxs_ap=idx_sb[:, call.off16: call.off16 + ncol * 8],
                            num_idxs=ncol * P,
                            num_idxs_reg=ncol * P,
                            elem_size=P,
                            single_packet=False,
                            queue_num=gq[0] % 4,
                        )
                        gq[0] += 1
                        if AGG_MODE < 2:
                            nc.vector.tensor_copy(
                                s_sb[:, R * F_DIM: R * F_DIM + F_DIM],
                                gt[:, 0:F_DIM])
                            continue
                        # both parity masks in one DVE op:
                        # mt[:, (2c+par)*P : ...] = [segid[:, 2(col)+par] == iota]
                        mt = mp.tile([P, 2 * MC * P], FP16, tag="mt")
                        nc.vector.tensor_tensor(
                            mt[:, :2 * ncol * P].rearrange(
                                "p (c k) -> p c k", c=2 * ncol, k=P),
                            segid_sb[:, 2 * call.col_off:
                                     2 * (call.col_off + ncol)]
                            .unsqueeze(2).broadcast_to([P, 2 * ncol, P]),
                            iota_sb[:, :].unsqueeze(1).broadcast_to(
                                [P, 2 * ncol, P]),
                            op=mybir.AluOpType.is_equal,
                        )
                        if AGG_MODE == 2:
                            nc.vector.tensor_copy(
                                s_sb[:, R * F_DIM: R * F_DIM + F_DIM],
                                mt[:, 0:F_DIM])
                            continue
                        for (ci, gl, par, st, sp) in call.mms:
                            nc.tensor.matmul(
                                ps[:, gl * F_DIM:(gl + 1) * F_DIM],
                                lhsT=mt[:, (2 * ci + par) * P:
                                        (2 * ci + par) * P + P],
                                rhs=gt[:, ci * P + par * F_DIM:
                                       ci * P + (par + 1) * F_DIM],
                                start=st, stop=sp)
                    if AGG_MODE < 3:
                        continue
                    sl = s_sb[:, R * RG * F_DIM: R * RG * F_DIM + rgw * F_DIM]
                    if w == 0:
                        nc.vector.tensor_copy(sl, ps[:, :rgw * F_DIM])
                    else:
                        nc.vector.tensor_tensor(sl, sl, ps[:, :rgw * F_DIM],
                                                op=mybir.AluOpType.add)
                if AGG_MODE < 3:
                    nc.vector.memset(s_sb[:, :], 0.0)
                return s_sb

            def finalize(s_sb, g_sb, b_sb, out_tag):
                """relu(dinv*(s + g) + b); destroys s_sb and g_sb."""
                nc.vector.tensor_tensor(s_sb[:, :], s_sb[:, :], g_sb[:, :],
                                        op=mybir.AluOpType.add)
                nc.vector.tensor_tensor(g_sb[:, :], s_sb[:, :],
                                        dinv_rep[:, :],
                                        op=mybir.AluOpType.mult)
                nc.vector.tensor_tensor(
                    s_sb[:, :].rearrange("p (g f) -> p g f", g=NG, f=F_DIM),
                    g_sb[:, :].rearrange("p (g f) -> p g f", g=NG, f=F_DIM),
                    b_sb[:, :].unsqueeze(1).broadcast_to([P, NG, F_DIM]),
                    op=mybir.AluOpType.add,
                )
                act = pers.tile([P, NF], FP16, tag=out_tag)
                nc.scalar.activation(act[:, :], s_sb[:, :],
                                     mybir.ActivationFunctionType.Relu)
                return act

            def pipeline(gsh2, gfull2):
                # ================= layer 1 =================
                h1pre = dense_matmul_stream(W1_sb, "hpre")
                g1_sb = mul_rep(h1pre, "gsb")
                table_exchange(g1_sb, gsh2[0], gfull2[0])
                s1 = aggregate(gfull2[0], "s")
                h1 = finalize(s1, g1_sb, b1_sb, "h1")

                # ================= layer 2 =================
                h2pre = dense_matmul_nodemajor(h1, W2_sb, F_DIM, "hpre")
                g2_sb = mul_rep(h2pre, "gsb")
                table_exchange(g2_sb, gsh2[1], gfull2[1])
                s2 = aggregate(gfull2[1], "s")
                h2a = finalize(s2, g2_sb, b2_sb, "gsb")
                h2 = pers.tile([P, NF], FP16, tag="hpre")
                nc.vector.tensor_tensor(h2[:, :], h2a[:, :], h1[:, :],
                                        op=mybir.AluOpType.add)

                # ================= head + log_softmax =================
                y_sb = dense_matmul_nodemajor(h2, Wl_sb, C_DIM, "ysb", F32)
                NC_ = NG * C_DIM
                yb = workp.tile([P, NC_], F32, tag="lsm", bufs=3)
                nc.vector.tensor_tensor(
                    yb[:, :].rearrange("p (g f) -> p g f", g=NG, f=C_DIM),
                    y_sb[:, :].rearrange("p (g f) -> p g f", g=NG, f=C_DIM),
                    bl_sb[:, :].unsqueeze(1).broadcast_to([P, NG, C_DIM]),
                    op=mybir.AluOpType.add,
                )
                rmax = workp.tile([P, NG], F32, tag="red", bufs=3)
                nc.vector.tensor_reduce(
                    rmax[:, :],
                    yb[:, :].rearrange("p (g f) -> p g f", g=NG, f=C_DIM),
                    axis=mybir.AxisListType.X, op=mybir.AluOpType.max,
                )
                tsub = workp.tile([P, NC_], F32, tag="lsm", bufs=3)
                nc.vector.tensor_tensor(
                    tsub[:, :].rearrange("p (g f) -> p g f", g=NG, f=C_DIM),
                    yb[:, :].rearrange("p (g f) -> p g f", g=NG, f=C_DIM),
                    rmax[:, :].unsqueeze(2).broadcast_to([P, NG, C_DIM]),
                    op=mybir.AluOpType.subtract,
                )
                e_sb = workp.tile([P, NC_], F32, tag="lsm", bufs=3)
                nc.scalar.activation(e_sb[:, :], tsub[:, :],
                                     mybir.ActivationFunctionType.Exp)
                ssum = workp.tile([P, NG], F32, tag="red", bufs=3)
                nc.vector.tensor_reduce(
                    ssum[:, :],
                    e_sb[:, :].rearrange("p (g f) -> p g f", g=NG, f=C_DIM),
                    axis=mybir.AxisListType.X, op=mybir.AluOpType.add,
                )
                lse = workp.tile([P, NG], F32, tag="red", bufs=3)
                nc.scalar.activation(lse[:, :], ssum[:, :],
                                     mybir.ActivationFunctionType.Ln)
                yout = workp.tile([P, NC_], F32, tag="lsm", bufs=3)
                nc.vector.tensor_tensor(
                    yout[:, :].rearrange("p (g f) -> p g f", g=NG, f=C_DIM),
                    tsub[:, :].rearrange("p (g f) -> p g f", g=NG, f=C_DIM),
                    lse[:, :].unsqueeze(2).broadcast_to([P, NG, C_DIM]),
                    op=mybir.AluOpType.subtract,
                )
                nc.sync.dma_start(
                    out=y_d[:, :].rearrange("(p g) f -> p (g f)", p=P),
                    in_=yout[:, :],
                )

            for r in range(repeat):
                pipeline(gsh_all[r], gfull_all[r])

    nc.compile()
    return nc


_CACHE = {}

LAST_RESULT = None


def kernel(x, edge_index, W1, b1, W2, b2, Wl, bl):
    global LAST_RESULT
    x = np.asarray(x)
    edge_index = np.asarray(edge_index)
    n_nodes = x.shape[0]
    key = (n_nodes, edge_index.shape[1],
           bytes(np.asarray(edge_index[1, :64]).astype(np.int64)))
    if key not in _CACHE:
        plan = build_plan(edge_index, n_nodes)
        nc = build_bass(plan)
        _CACHE[key] = (plan, nc)
    plan, nc = _CACHE[key]

    in_maps = build_inputs(plan, x, np.asarray(W1), np.asarray(b1),
                           np.asarray(W2), np.asarray(b2),
                           np.asarray(Wl), np.asarray(bl))
    res = run_bass_kernel_spmd(nc, in_maps, core_ids=list(range(N_CORES)),
                               trace=False)
    LAST_RESULT = res
    y = np.empty((n_nodes, C_DIM), dtype=np.float32)
    for c in range(N_CORES):
        nl = plan.nodes_per_core[c]
        ranks = plan.usable[: len(nl)]
        yc = res.results[c]["y"]   # row index = pos = (r%128)*NG + r//128
        posn = (ranks % P) * NG + ranks // P
        y[nl] = yc[posn]
    return y


# revision 9
# speedup vs baseline: 1.0850x; 1.0850x over previous
"""GCN (2-layer + linear head + log_softmax) on 8 Trainium2 NeuronCores.

Strategy (graph/data parallel, per sharding hint):
  - Nodes partitioned across 8 cores (degree-sorted serpentine), weights
    replicated.  Per GCN layer each core computes h = x_shard @ W on PE,
    scales rows by dinv = (deg+1)^-1/2 and AllGathers the scaled table in
    2 window chunks of PAIR rows: one 256B table row holds the features
    of two nodes (adjacent feature groups 2G,2G+1 at the same partition),
    so the exchange ships each node's 128B of features exactly once.
  - Aggregation: batched SWDGE dma_gather (mlp gpsimd library) pulls
    per-edge 256B pair rows (int16 window-local indices, 4 SWDGE queues
    round-robin) into SBUF column tiles; PE one-hot "segment matmuls"
    reduce each 128-slot column into per-destination partial sums in
    PSUM.  Each column gets two matmuls - parity A (even source group,
    first 128B of the row) and parity B (odd, second 128B) - with masks
    M[e, rank] = [segid_par[e] == rank] built by DVE from per-parity
    segid tables; a sentinel segid kills padding slots, so padding
    indices are spread over the table (a single shared zero row would
    hotspot one HBM line and serialize the SDMA engines).
  - Math identity:  out[d] = dinv[d] * sum_{e:dst=d} dinv[src]*h[src]
                             + dinv[d]^2 * h[d] + b
    so the gathered table is pre-scaled by dinv and no per-edge
    coefficients are needed.

Host-side numpy does only graph-structure preprocessing (degree counting,
node->core/rank assignment, gather-index/segment-id construction) and the
output unpermute.  All floating-point tensor math runs on the NeuronCores.
"""

import os

import numpy as np

import concourse.bass as bass
import concourse.bacc as bacc
import concourse.mybir as mybir
import concourse.tile as tile
from concourse.bass_utils import run_bass_kernel_spmd
from concourse.masks import make_identity
from concourse import library_config

FP16 = mybir.dt.float16
F32 = mybir.dt.float32
I16 = mybir.dt.int16

N_CORES = 8
P = 128           # partitions
F_DIM = 64        # in = hidden = 64
C_DIM = 16
NG = 98           # feature groups per core (SHR / 128)
SHR = 12544       # node slots per core (128 * 98)
W = 2             # gather windows (int16 index range)
WP = 64           # partitions per window (128 / 2)
PG = NG // 2      # pair groups per core (49): row = groups (2G, 2G+1)
WG = WP * PG      # pair rows contributed per core per window = 3136
WROWS = N_CORES * WG   # pair rows per window table = 25088
RG = 8            # dst groups per PSUM range
NR = (NG + RG - 1) // RG   # 13 ranges
CAP = int(os.environ.get("KCAP", "16"))   # max gather columns per call
                  # (small calls round-robined over 4 SWDGE queues)
SEG_PAD = 200.0   # segid sentinel for padding slots (matches no rank)


class _Call:
    __slots__ = ("w", "ncols", "off16", "col_off", "mms")


class _Plan:
    pass


def build_plan(edge_index, n_nodes):
    """Host-side graph preprocessing.  Pure index math, O(E log E)."""
    src = np.asarray(edge_index[0], dtype=np.int64)
    dst = np.asarray(edge_index[1], dtype=np.int64)
    E = src.shape[0]

    deg = np.bincount(dst, minlength=n_nodes).astype(np.int64)

    # serpentine degree-desc core assignment
    order = np.argsort(-deg, kind="stable")
    pos = np.arange(n_nodes)
    blk, lane = pos // N_CORES, pos % N_CORES
    core_of_pos = np.where(blk % 2 == 0, lane, N_CORES - 1 - lane)
    nodes_per_core = [order[core_of_pos == c] for c in range(N_CORES)]
    n_max = max(len(v) for v in nodes_per_core)
    assert n_max <= SHR, "shard overflow"

    usable = np.arange(SHR, dtype=np.int64)

    rank_of = np.full(n_nodes, -1, dtype=np.int64)
    core_of = np.full(n_nodes, -1, dtype=np.int64)
    for c, nl in enumerate(nodes_per_core):
        rank_of[nl] = usable[: len(nl)]
        core_of[nl] = c

    p_of = rank_of % P
    g_of = rank_of // P
    w_of = p_of // WP
    par_of = g_of % 2
    # window-local pair-row of a node
    loc_of = core_of * WG + (p_of % WP) * PG + g_of // 2

    # per-core per (dst-group, src-window) edge counts
    cnt = np.zeros((N_CORES, NG, W), dtype=np.int64)
    ecore = core_of[dst]
    for c in range(N_CORES):
        m = ecore == c
        np.add.at(cnt[c], (g_of[dst[m]], w_of[src[m]]), 1)

    # shared (max over cores) column counts per (g, w); >=1 so every group
    # appears in window 0 (s init via copy) and DVE accum stays coarse
    cols_gw = np.maximum(1, -(-cnt.max(axis=0) // P))   # [NG, W]

    # global column offsets, order (w, range, g)
    o_gw = np.zeros((NG, W), dtype=np.int64)
    col = 0
    rw_list = []   # (w, R, rgw, [calls])
    for w in range(W):
        for R in range(NR):
            g0, g1 = R * RG, min(NG, R * RG + RG)
            span0 = col
            for g in range(g0, g1):
                o_gw[g, w] = col
                col += int(cols_gw[g, w])
            # split span into calls of <= CAP columns
            calls = []
            c0 = span0
            while c0 < col:
                c1 = min(col, c0 + CAP)
                call = _Call()
                call.w = w
                call.ncols = c1 - c0
                call.off16 = c0 * 8          # slot offset / 16
                call.col_off = c0
                call.mms = []
                calls.append(call)
                c0 = c1
            # per-column matmul descriptors: two per column (parity A/B)
            for g in range(g0, g1):
                kc = int(cols_gw[g, w])
                for k in range(kc):
                    cg = int(o_gw[g, w]) + k
                    for call in calls:
                        if call.col_off <= cg < call.col_off + call.ncols:
                            ci = cg - call.col_off
                            gl = g - g0
                            for par in (0, 1):
                                call.mms.append(
                                    (ci, gl, par,
                                     k == 0 and par == 0,
                                     k == kc - 1 and par == 1))
                            break
            rw_list.append((w, R, g1 - g0, calls))
    TOT_COLS = col
    TOT_SLOTS = TOT_COLS * P
    maxcall = max(c.ncols for (_, _, _, cl) in rw_list for c in cl)

    # per-core gather indices + per-parity segment ids
    idx16 = np.empty((N_CORES, P, TOT_SLOTS // 16), dtype=np.int16)
    segid = np.empty((N_CORES, P, 2 * TOT_COLS), dtype=np.float16)
    for c in range(N_CORES):
        m = ecore == c
        d_g = g_of[dst[m]]
        d_p = p_of[dst[m]]
        s_w = w_of[src[m]]
        s_loc = loc_of[src[m]]
        s_par = par_of[src[m]]
        key = d_g * W + s_w
        o = np.argsort(key, kind="stable")
        key_s = key[o]
        first = np.searchsorted(key_s, key_s, side="left")
        k = np.arange(len(key_s)) - first
        colno = o_gw[d_g[o], s_w[o]] + k // P
        part = k % P
        slot = colno * P + part
        # padding slots gather garbage rows the sentinel mask zeroes out;
        # spread them so no single 256B HBM line hotspots the SDMA engines
        idx_flat = ((np.arange(TOT_SLOTS) * 97) % WROWS).astype(np.int16)
        segAB = np.full((TOT_SLOTS, 2), SEG_PAD, dtype=np.float16)
        idx_flat[slot] = s_loc[o].astype(np.int16)
        segAB[slot, s_par[o]] = d_p[o].astype(np.float16)
        wrapped = idx_flat.reshape(TOT_SLOTS // 16, 16).T   # [16, S/16]
        idx16[c] = np.tile(wrapped, (8, 1))
        # [P, 2*TOT_COLS], parity interleaved per column
        segid[c] = segAB.reshape(TOT_COLS, P, 2).transpose(1, 0, 2).reshape(
            P, 2 * TOT_COLS)

    plan = _Plan()
    plan.n_nodes = n_nodes
    plan.E = E
    plan.TOT_COLS = TOT_COLS
    plan.TOT_SLOTS = TOT_SLOTS
    plan.maxcall = maxcall
    plan.rw_list = rw_list
    plan.nodes_per_core = nodes_per_core
    plan.rank_of = rank_of
    plan.usable = usable
    plan.idx16 = idx16
    plan.segid = segid
    plan.deg = deg
    return plan


def build_inputs(plan, x, W1, b1, W2, b2, Wl, bl):
    """Per-core input dicts for run_bass_kernel_spmd."""
    in_maps = []
    W1h = np.ascontiguousarray(W1.astype(np.float16))
    W2h = np.ascontiguousarray(W2.astype(np.float16))
    Wlh = np.ascontiguousarray(Wl.astype(np.float16))
    b1r = np.ascontiguousarray(np.broadcast_to(b1.astype(np.float16), (P, F_DIM)))
    b2r = np.ascontiguousarray(np.broadcast_to(b2.astype(np.float16), (P, F_DIM)))
    blr = np.ascontiguousarray(np.broadcast_to(bl.astype(np.float32), (P, C_DIM)))
    iota = np.ascontiguousarray(
        np.broadcast_to(np.arange(P, dtype=np.float16), (P, P)))
    for c in range(N_CORES):
        nl = plan.nodes_per_core[c]
        ranks = plan.usable[: len(nl)]
        xT = np.zeros((F_DIM, SHR), dtype=np.float16)
        xT[:, ranks] = np.asarray(x)[nl].astype(np.float16).T
        degp1 = np.full((P, NG), 1e30, dtype=np.float32)
        degp1[ranks % P, ranks // P] = (plan.deg[nl] + 1).astype(np.float32)
        in_maps.append({
            "xT": xT,
            "degp1": degp1,
            "gidx": np.ascontiguousarray(plan.idx16[c]),
            "segid": np.ascontiguousarray(plan.segid[c]),
            "iota": iota,
            "W1": W1h, "W2": W2h, "Wl": Wlh,
            "b1r": b1r, "b2r": b2r, "blr": blr,
        })
    return in_maps


def build_bass(plan, repeat=1):
    NF = NG * F_DIM
    TOT16 = plan.TOT_SLOTS // 16
    MC = plan.maxcall
    nc = bacc.Bacc("TRN2", target_bir_lowering=False, debug=False,
                   num_devices=N_CORES, dynamic_dma_scratch_size=16384,
                   num_swdge_queues=4)

    xT_d = nc.dram_tensor("xT", [F_DIM, SHR], FP16, kind="ExternalInput")
    degp1_d = nc.dram_tensor("degp1", [P, NG], F32, kind="ExternalInput")
    gidx_d = nc.dram_tensor("gidx", [P, TOT16], I16, kind="ExternalInput")
    segid_d = nc.dram_tensor("segid", [P, 2 * plan.TOT_COLS], FP16,
                             kind="ExternalInput")
    iota_d = nc.dram_tensor("iota", [P, P], FP16, kind="ExternalInput")
    W1_d = nc.dram_tensor("W1", [F_DIM, F_DIM], FP16, kind="ExternalInput")
    W2_d = nc.dram_tensor("W2", [F_DIM, F_DIM], FP16, kind="ExternalInput")
    Wl_d = nc.dram_tensor("Wl", [F_DIM, C_DIM], FP16, kind="ExternalInput")
    b1r_d = nc.dram_tensor("b1r", [P, F_DIM], FP16, kind="ExternalInput")
    b2r_d = nc.dram_tensor("b2r", [P, F_DIM], FP16, kind="ExternalInput")
    blr_d = nc.dram_tensor("blr", [P, C_DIM], F32, kind="ExternalInput")
    y_d = nc.dram_tensor("y", [SHR, C_DIM], F32, kind="ExternalOutput")

    gsh_all = [[[nc.dram_tensor(f"gsh{l}_{w}_{r}", [WG, 2 * F_DIM], FP16)
                 for w in range(W)] for l in range(2)] for r in range(repeat)]
    gfull_all = [[[nc.dram_tensor(f"gfull{l}_{w}_{r}", [WROWS, 2 * F_DIM],
                                  FP16, addr_space="Shared")
                   for w in range(W)] for l in range(2)] for r in range(repeat)]

    rg = [list(range(N_CORES))]

    with tile.TileContext(nc) as tc:
        with (
            tc.tile_pool(name="const", bufs=1) as constp,
            tc.tile_pool(name="persist", bufs=1) as pers,
            tc.tile_pool(name="work", bufs=2) as workp,
            tc.tile_pool(name="xt", bufs=2) as xtp,
            tc.tile_pool(name="gath", bufs=int(os.environ.get("KGB", "8"))) as gathp,
            tc.tile_pool(name="mm", bufs=int(os.environ.get("KMB", "4"))) as mp,
            tc.tile_pool(name="psum", bufs=int(os.environ.get("KPB", "2")), space="PSUM") as psump,
        ):
            # Load the mlp gpsimd library (dma_gather) up front: a reload
            # auto-inserted mid-program next to in-flight collectives kills
            # the Q7 cores (NRT_EXEC_UNIT_UNRECOVERABLE).
            nc.gpsimd.load_library(library_config.mlp)

            # ---- constants ----
            W1_sb = constp.tile([F_DIM, F_DIM], FP16, tag="W1")
            nc.sync.dma_start(out=W1_sb, in_=W1_d[:, :])
            W2_sb = constp.tile([F_DIM, F_DIM], FP16, tag="W2")
            nc.sync.dma_start(out=W2_sb, in_=W2_d[:, :])
            Wl_sb = constp.tile([F_DIM, C_DIM], FP16, tag="Wl")
            nc.sync.dma_start(out=Wl_sb, in_=Wl_d[:, :])
            b1_sb = constp.tile([P, F_DIM], FP16, tag="b1")
            nc.sync.dma_start(out=b1_sb, in_=b1r_d[:, :])
            b2_sb = constp.tile([P, F_DIM], FP16, tag="b2")
            nc.sync.dma_start(out=b2_sb, in_=b2r_d[:, :])
            bl_sb = constp.tile([P, C_DIM], F32, tag="bl")
            nc.sync.dma_start(out=bl_sb, in_=blr_d[:, :])
            ident = constp.tile([P, P], FP16, tag="ident")
            make_identity(nc, ident[:, :])
            iota_sb = constp.tile([P, P], FP16, tag="iota")
            nc.sync.dma_start(out=iota_sb, in_=iota_d[:, :])

            idx_sb = constp.tile([P, TOT16], I16, tag="idx")
            nc.sync.dma_start(out=idx_sb, in_=gidx_d[:, :])
            segid_sb = constp.tile([P, 2 * plan.TOT_COLS], FP16, tag="segid")
            nc.sync.dma_start(out=segid_sb, in_=segid_d[:, :])

            # ---- dinv ----
            degp1_sb = constp.tile([P, NG], F32, tag="degp1")
            nc.sync.dma_start(out=degp1_sb, in_=degp1_d[:, :])
            rec_sb = constp.tile([P, NG], F32, tag="rec")
            nc.vector.reciprocal(rec_sb[:, :], degp1_sb[:, :])
            dinv_sb = constp.tile([P, NG], F32, tag="dinv")
            nc.scalar.activation(dinv_sb[:, :], rec_sb[:, :],
                                 mybir.ActivationFunctionType.Sqrt)
            dinv_rep = constp.tile([P, NF], FP16, tag="dinvrep")
            nc.vector.tensor_copy(
                dinv_rep[:, :].rearrange("p (g f) -> p g f", g=NG, f=F_DIM),
                dinv_sb[:, :].unsqueeze(2).broadcast_to([P, NG, F_DIM]),
            )

            def dense_matmul_stream(W_sb, out_tag):
                """layer-1 shard matmul, xT streamed from DRAM."""
                out_sb = pers.tile([P, NF], FP16, tag=out_tag)
                per_ps = 8
                for blk0 in range(0, NG, per_ps):
                    blk1 = min(NG, blk0 + per_ps)
                    nb = blk1 - blk0
                    xt = xtp.tile([F_DIM, per_ps * P], FP16, tag="xt")
                    nc.sync.dma_start(out=xt[:, :nb * P],
                                      in_=xT_d[:, blk0 * P: blk1 * P])
                    ps = psump.tile([P, 512], F32, tag="mmps")
                    for g in range(blk0, blk1):
                        kk = g - blk0
                        nc.tensor.matmul(
                            ps[:, kk * F_DIM:(kk + 1) * F_DIM],
                            lhsT=xt[:, kk * P: kk * P + P],
                            rhs=W_sb[:, :], start=True, stop=True)
                    nc.scalar.activation(
                        out_sb[:, blk0 * F_DIM: blk0 * F_DIM + nb * F_DIM],
                        ps[:, :nb * F_DIM],
                        mybir.ActivationFunctionType.Copy)
                return out_sb

            def dense_matmul_nodemajor(h_sb, W_sb, n_out, out_tag,
                                       out_dtype=FP16):
                """input node-major [128, NG*64]: PE-transpose 4 groups at a
                time, then matmul."""
                out_sb = pers.tile([P, NG * n_out], out_dtype, tag=out_tag)
                for blk0 in range(0, NG, 4):
                    blk1 = min(NG, blk0 + 4)
                    nb = blk1 - blk0
                    tps = psump.tile([F_DIM, 512], FP16, tag="tps")
                    for g in range(blk0, blk1):
                        kk = g - blk0
                        nc.tensor.transpose(
                            tps[:, kk * P: kk * P + P],
                            in_=h_sb[:, g * F_DIM: (g + 1) * F_DIM],
                            identity=ident[:, :],
                        )
                    hTt = workp.tile([F_DIM, 512], FP16, tag="hTt")
                    nc.scalar.activation(hTt[:, :nb * P], tps[:, :nb * P],
                                         mybir.ActivationFunctionType.Copy)
                    ps = psump.tile([P, 512], F32, tag="mmps")
                    for g in range(blk0, blk1):
                        kk = g - blk0
                        nc.tensor.matmul(
                            ps[:, kk * n_out:(kk + 1) * n_out],
                            lhsT=hTt[:, kk * P: kk * P + P],
                            rhs=W_sb[:, :], start=True, stop=True)
                    nc.scalar.activation(
                        out_sb[:, blk0 * n_out: blk0 * n_out + nb * n_out],
                        ps[:, :nb * n_out],
                        mybir.ActivationFunctionType.Copy)
                return out_sb

            def mul_rep(h_sb, tag):
                o = pers.tile([P, NF], FP16, tag=tag)
                nc.vector.tensor_tensor(o[:, :], h_sb[:, :], dinv_rep[:, :],
                                        op=mybir.AluOpType.mult)
                return o

            def cc_observe(gfull_t):
                # tiny SWDGE read carries the collective-done wait once
                obs = constp.tile([1, F_DIM], FP16, tag="ccobs")
                nc.gpsimd.dma_start(out=obs[:, :], in_=gfull_t[0:1, 0:F_DIM])

            KCC = int(os.environ.get("KCC", "1"))

            def table_exchange(g_sb, gsh, gfull):
                """write dinv-scaled shard as pair rows (2 nodes / 256B) and
                AllGather per window.  Pair layout is free: groups 2G,2G+1
                are adjacent in the free dim of g_sb."""
                for w in range(W):
                    nc.sync.dma_start(
                        out=gsh[w][:, :].rearrange("(q G) f -> q (G f)", q=WP),
                        in_=g_sb[WP * w: WP * (w + 1), :])
                    if not KCC:
                        continue
                    nc.gpsimd.collective_compute(
                        "AllGather", mybir.AluOpType.bypass,
                        replica_groups=rg,
                        ins=[gsh[w][:, :].opt()], outs=[gfull[w][:, :].opt()],
                    )
                    cc_observe(gfull[w])

            AGG_MODE = int(os.environ.get("KAGG", "3"))

            gq = [0]  # global gather queue round-robin

            def aggregate(gfull, out_tag):
                s_sb = pers.tile([P, NF], FP16, tag=out_tag)
                if AGG_MODE == 0:
                    nc.vector.memset(s_sb[:, :], 0.0)
                    return s_sb
                for (w, R, rgw, calls) in plan.rw_list:
                    if AGG_MODE >= 3:
                        ps = psump.tile([P, 512], F32, tag="agg")
                    for call in calls:
                        ncol = call.ncols
                        gt = gathp.tile([P, MC * P], FP16, tag="gt")
                        nc.gpsimd.dma_gather(
                            out_ap=gt[:, :ncol * P].rearrange(
                                "p (c e) -> p c e", c=ncol, e=P),
                            in_ap=gfull[w][:, :],
                            idxs_ap=idx_sb[:, call.off16: call.off16 + ncol * 8],
                            num_idxs=ncol * P,
                            num_idxs_reg=ncol * P,
                            elem_size=P,
                            single_packet=False,
                            queue_num=gq[0] % 4,
                        )
                        gq[0] += 1
                        if AGG_MODE < 2:
                            nc.vector.tensor_copy(
                                s_sb[:, R * F_DIM: R * F_DIM + F_DIM],
                                gt[:, 0:F_DIM])
                            continue
                        # both parity masks in one DVE op:
                        # mt[:, (2c+par)*P : ...] = [segid[:, 2(col)+par] == iota]
                        mt = mp.tile([P, 2 * MC * P], FP16, tag="mt")
                        nc.vector.tensor_tensor(
                            mt[:, :2 * ncol * P].rearrange(
                                "p (c k) -> p c k", c=2 * ncol, k=P),
                            segid_sb[:, 2 * call.col_off:
                                     2 * (call.col_off + ncol)]
                            .unsqueeze(2).broadcast_to([P, 2 * ncol, P]),
                            iota_sb[:, :].unsqueeze(1).broadcast_to(
                                [P, 2 * ncol, P]),
                            op=mybir.AluOpType.is_equal,
                        )
                        if AGG_MODE == 2:
                            nc.vector.tensor_copy(
                                s_sb[:, R * F_DIM: R * F_DIM + F_DIM],
                                mt[:, 0:F_DIM])
                            continue
                        for (ci, gl, par, st, sp) in call.mms:
                            nc.tensor.matmul(
                                ps[:, gl * F_DIM:(gl + 1) * F_DIM],
                                lhsT=mt[:, (2 * ci + par) * P:
                                        (2 * ci + par) * P + P],
                                rhs=gt[:, ci * P + par * F_DIM:
                                       ci * P + (par + 1) * F_DIM],
                                start=st, stop=sp)
                    if AGG_MODE < 3:
                        continue
                    sl = s_sb[:, R * RG * F_DIM: R * RG * F_DIM + rgw * F_DIM]
                    if w == 0:
                        nc.vector.tensor_copy(sl, ps[:, :rgw * F_DIM])
                    else:
                        nc.vector.tensor_tensor(sl, sl, ps[:, :rgw * F_DIM],
                                                op=mybir.AluOpType.add)
                if AGG_MODE < 3:
                    nc.vector.memset(s_sb[:, :], 0.0)
                return s_sb

            def finalize(s_sb, g_sb, b_sb, out_tag):
                """relu(dinv*(s + g) + b); destroys s_sb and g_sb."""
                nc.vector.tensor_tensor(s_sb[:, :], s_sb[:, :], g_sb[:, :],
                                        op=mybir.AluOpType.add)
                nc.vector.tensor_tensor(g_sb[:, :], s_sb[:, :],
                                        dinv_rep[:, :],
                                        op=mybir.AluOpType.mult)
                nc.vector.tensor_tensor(
                    s_sb[:, :].rearrange("p (g f) -> p g f", g=NG, f=F_DIM),
                    g_sb[:, :].rearrange("p (g f) -> p g f", g=NG, f=F_DIM),
                    b_sb[:, :].unsqueeze(1).broadcast_to([P, NG, F_DIM]),
                    op=mybir.AluOpType.add,
                )
                act = pers.tile([P, NF], FP16, tag=out_tag)
                nc.scalar.activation(act[:, :], s_sb[:, :],
                                     mybir.ActivationFunctionType.Relu)
                return act

            def pipeline(gsh2, gfull2):
                # ================= layer 1 =================
                h1pre = dense_matmul_stream(W1_sb, "hpre")
                g1_sb = mul_rep(h1pre, "gsb")
                table_exchange(g1_sb, gsh2[0], gfull2[0])
                s1 = aggregate(gfull2[0], "s")
                h1 = finalize(s1, g1_sb, b1_sb, "h1")

                # ================= layer 2 =================
                h2pre = dense_matmul_nodemajor(h1, W2_sb, F_DIM, "hpre")
                g2_sb = mul_rep(h2pre, "gsb")
                table_exchange(g2_sb, gsh2[1], gfull2[1])
                s2 = aggregate(gfull2[1], "s")
                h2a = finalize(s2, g2_sb, b2_sb, "gsb")
                h2 = pers.tile([P, NF], FP16, tag="hpre")
                nc.vector.tensor_tensor(h2[:, :], h2a[:, :], h1[:, :],
                                        op=mybir.AluOpType.add)

                # ================= head + log_softmax =================
                y_sb = dense_matmul_nodemajor(h2, Wl_sb, C_DIM, "ysb", F32)
                NC_ = NG * C_DIM
                yb = workp.tile([P, NC_], F32, tag="lsm", bufs=3)
                nc.vector.tensor_tensor(
                    yb[:, :].rearrange("p (g f) -> p g f", g=NG, f=C_DIM),
                    y_sb[:, :].rearrange("p (g f) -> p g f", g=NG, f=C_DIM),
                    bl_sb[:, :].unsqueeze(1).broadcast_to([P, NG, C_DIM]),
                    op=mybir.AluOpType.add,
                )
                rmax = workp.tile([P, NG], F32, tag="red", bufs=3)
                nc.vector.tensor_reduce(
                    rmax[:, :],
                    yb[:, :].rearrange("p (g f) -> p g f", g=NG, f=C_DIM),
                    axis=mybir.AxisListType.X, op=mybir.AluOpType.max,
                )
                tsub = workp.tile([P, NC_], F32, tag="lsm", bufs=3)
                nc.vector.tensor_tensor(
                    tsub[:, :].rearrange("p (g f) -> p g f", g=NG, f=C_DIM),
                    yb[:, :].rearrange("p (g f) -> p g f", g=NG, f=C_DIM),
                    rmax[:, :].unsqueeze(2).broadcast_to([P, NG, C_DIM]),
                    op=mybir.AluOpType.subtract,
                )
                e_sb = workp.tile([P, NC_], F32, tag="lsm", bufs=3)
                nc.scalar.activation(e_sb[:, :], tsub[:, :],
                                     mybir.ActivationFunctionType.Exp)
                ssum = workp.tile([P, NG], F32, tag="red", bufs=3)
                nc.vector.tensor_reduce(
                    ssum[:, :],
                    e_sb[:, :].rearrange("p (g f) -> p g f", g=NG, f=C_DIM),
                    axis=mybir.AxisListType.X, op=mybir.AluOpType.add,
                )
                lse = workp.tile([P, NG], F32, tag="red", bufs=3)
                nc.scalar.activation(lse[:, :], ssum[:, :],
                                     mybir.ActivationFunctionType.Ln)
                yout = workp.tile([P, NC_], F32, tag="lsm", bufs=3)
                nc.vector.tensor_tensor(
                    yout[:, :].rearrange("p (g f) -> p g f", g=NG, f=C_DIM),
                    tsub[:, :].rearrange("p (g f) -> p g f", g=NG, f=C_DIM),
                    lse[:, :].unsqueeze(2).broadcast_to([P, NG, C_DIM]),
                    op=mybir.AluOpType.subtract,
                )
                nc.sync.dma_start(
                    out=y_d[:, :].rearrange("(p g) f -> p (g f)", p=P),
                    in_=yout[:, :],
                )

            for r in range(repeat):
                pipeline(gsh_all[r], gfull_all[r])

    nc.compile()
    return nc


_CACHE = {}

LAST_RESULT = None


def kernel(x, edge_index, W1, b1, W2, b2, Wl, bl):
    global LAST_RESULT
    x = np.asarray(x)
    edge_index = np.asarray(edge_index)
    n_nodes = x.shape[0]
    key = (n_nodes, edge_index.shape[1],
           bytes(np.asarray(edge_index[1, :64]).astype(np.int64)))
    if key not in _CACHE:
        plan = build_plan(edge_index, n_nodes)
        nc = build_bass(plan)
        _CACHE[key] = (plan, nc)
    plan, nc = _CACHE[key]

    in_maps = build_inputs(plan, x, np.asarray(W1), np.asarray(b1),
                           np.asarray(W2), np.asarray(b2),
                           np.asarray(Wl), np.asarray(bl))
    res = run_bass_kernel_spmd(nc, in_maps, core_ids=list(range(N_CORES)),
                               trace=False)
    LAST_RESULT = res
    y = np.empty((n_nodes, C_DIM), dtype=np.float32)
    for c in range(N_CORES):
        nl = plan.nodes_per_core[c]
        ranks = plan.usable[: len(nl)]
        yc = res.results[c]["y"]   # row index = pos = (r%128)*NG + r//128
        posn = (ranks % P) * NG + ranks // P
        y[nl] = yc[posn]
    return y
